# revision 33
# baseline (speedup 1.0000x reference)
"""Trainium2 Bass kernel for the nn_Attention problem (non-local attention).

Reference (per batch b, N = W*H positions):
    q = wq@r + bq; k = wk@i + bk; corr[n,m] = q_n . k_m
    attn = softmax_m(corr); v = wv@i + bv
    out = v @ attn^T; y = relu(BN(wc@out + bc)); result = img + y

Algebraic scheme (P-scheme, K=128 contraction — keeps the PE HAM-warm;
a K=64 direct q^T k variant measured cold/half-clock):
    corr[n,m] = r_n^T A i_m + u[m] + row-const,  A = wq^T wk
  - row-consts cancel in softmax (dropped); u[m] = (wk^T bq) . i_m is
    folded into vhat as a per-key e^u scale.
  - P = A @ i on device ([C,N] fp16); logits tile = P_tile^T @ r.
  - max-subtraction skipped: |logit| < ~55 << 88 (bf16 exp range).
  - Vhat = [v * e^u ; e^u]; po = Vhat^T @ exp(logits); row 64 = softmax
    denominator. bv and BN fold into the output projection/bias on host.
  - Residual: img columns are ROTATED per-core on host so the query half
    always sits at columns 0:NQ (attention is permutation-invariant over
    keys) -> one program for all 8 cores, no separate fp32 residual input.

Precision: logits path fp16, attention-value path bf16, normalization fp32.
Sharding: 8 cores = 4 batches x 2 query halves; full keys per core.

Perf structure (ACT exp stream is the bottleneck: 52 exps, ~69us):
  - input DMA split across SP + ACT hwdge queues and the Pool swdge queue,
    row-split transfers ordered so P-gen starts right after engine bringup
    (PE never idles a HAM window); output DMA overlaps per chunk.
  - dummy exp preloads the ACT exp table during the DMA wait; PE warmup
    matmuls bridge the DMA wait to keep the HAM clock gate open.
  - wide exps amortize ACT per-instr overhead. PSUM: asymmetric pc pools
    (1536-wide x1 + 1024-wide x1, alternating groups) + po x2 + scratch
    = 8 banks; double-buffered po removes every chunk-boundary hazard.
  - FLAT software pipeline over all 52 (chunk, group) pairs: pc/exp lead,
    AV matmuls lag 3 groups, postamble ops spread one-per-group so no PE
    instruction ever waits on the 3.3us DVE reciprocal.
  - second-half P/vhat generation is deferred into chunk 0's groups via
    the scratch bank, so the exp stream starts ~6us after bringup.
  - tail: final postamble runs as two pipelined 256-column halves.
"""

import numpy as np

BN_EPS = 1e-5

_CACHE: dict = {}
_LAST_RESULTS = None  # BassKernelResults of the most recent run (for profiling)


def _build_program(C: int, N: int, NQ: int, Co: int):
    import concourse.bass as bass
    import concourse.tile as tile
    from concourse import bacc, mybir

    f32 = mybir.dt.float32
    f16 = mybir.dt.float16
    bf16 = mybir.dt.bfloat16
    Exp = mybir.ActivationFunctionType.Exp
    Relu = mybir.ActivationFunctionType.Relu
    Add = mybir.AluOpType.add
    Max = mybir.AluOpType.max

    MT = N // 128        # key tiles (32)
    NCH = NQ // 512      # query chunks per core (4)
    # groups alternate 3/2 key tiles (1536/1024-wide exps): 13 per chunk
    GRPS = []
    t = 0
    while t < MT:
        cnt = 3 if (len(GRPS) % 2 == 0 and t + 3 <= MT) else min(2, MT - t)
        GRPS.append((t, cnt))
        t += cnt
    NG = len(GRPS)       # 13
    LAG = 3

    nc = bacc.Bacc()
    w16_d = nc.declare_dram_parameter("w16", [C, C + Co + 1], f16, isOutput=False)
    img_d = nc.declare_dram_parameter("img16", [C, N], f16, isOutput=False)
    rng_d = nc.declare_dram_parameter("rng16", [C, NQ], f16, isOutput=False)
    wbf_d = nc.declare_dram_parameter("wbf", [Co, C], bf16, isOutput=False)
    x32_d = nc.declare_dram_parameter("x32", [C, 1], f32, isOutput=False)
    out_d = nc.declare_dram_parameter("out", [C, NQ], f32, isOutput=True)

    with tile.TileContext(nc) as tc:
        with (
            tc.tile_pool(name="const", bufs=1) as cpool,
            tc.tile_pool(name="work", bufs=4) as wpool,
            tc.tile_pool(name="et", bufs=2 * NG) as epool,
            tc.tile_pool(name="ps_c3", bufs=1, space="PSUM") as ps_c3,
            tc.tile_pool(name="ps_c2", bufs=1, space="PSUM") as ps_c2,
            tc.tile_pool(name="ps_o", bufs=2, space="PSUM") as ps_o,
            tc.tile_pool(name="ps_x", bufs=1, space="PSUM") as ps_x,
        ):
            img_t = cpool.tile([C, N], f16)
            rng_t = cpool.tile([C, NQ], f16)
            w16_t = cpool.tile([C, C + Co + 1], f16)
            wbf_t = cpool.tile([Co, C], bf16)
            x32_t = cpool.tile([C, 1], f32)
            at_t = w16_t[:, 0:C]                 # A^T = wk^T wq
            wvg_t = w16_t[:, C:C + Co + 1]       # [wv^T | wk^T bq]
            bcc_t = x32_t[:, 0:1]

            # ---- input DMAs: SP + ACT hwdge queues + Pool swdge ----
            # img q1 leads on both hw queues (gates P-gen); tiny rng chunk 0
            # second (gates the first exp); weights ride the Pool swdge.
            H2 = C // 2
            nc.sync.dma_start(w16_t[:], w16_d[:])
            nc.scalar.dma_start(img_t[:, 1024:2048], img_d[:, 1024:2048])
            nc.sync.dma_start(img_t[:, 0:1024], img_d[:, 0:1024])
            nc.gpsimd.dma_start(x32_t[:], x32_d[:])
            nc.sync.dma_start(rng_t[:, 0:512], rng_d[:, 0:512])
            nc.gpsimd.dma_start(img_t[0:H2, 2048:3072], img_d[0:H2, 2048:3072])
            nc.gpsimd.dma_start(img_t[H2:C, 2048:3072], img_d[H2:C, 2048:3072])
            nc.scalar.dma_start(rng_t[:, 512:NQ], rng_d[:, 512:NQ])
            nc.gpsimd.dma_start(img_t[0:H2, 3072:N], img_d[0:H2, 3072:N])
            nc.gpsimd.dma_start(img_t[H2:C, 3072:N], img_d[H2:C, 3072:N])
            nc.scalar.dma_start(wbf_t[:], wbf_d[:])

            p_t = cpool.tile([C, N], f16)            # P = A @ img
            vhat_t = cpool.tile([128, MT, Co + 1], bf16)
            eu_t = cpool.tile([128, MT], f32)        # e^u per key
            fin_t = cpool.tile([C, NQ], f32)
            ones_t = cpool.tile([1, Co], bf16)
            dummy_t = cpool.tile([1, 2], f32)
            warm_t = cpool.tile([C, 512], f16)

            nc.vector.memset(ones_t[:], 1.0)
            nc.vector.memset(dummy_t[:], 0.0)
            nc.vector.memset(warm_t[:], 0.0)
            # preload the exp table while DMAs run
            nc.scalar.activation(dummy_t[0:1, 1:2], dummy_t[0:1, 0:1], Exp)

            # PE warmup during the DMA wait (HAM needs ~3.4us of activity)
            warm_ps = ps_o.tile([Co + 1, 512], f32, tag="po")
            for _ in range(7):
                nc.tensor.matmul(warm_ps[:], warm_t[:, 0:Co + 1],
                                 warm_t[:], start=True, stop=True)

            # ---- P / vhat generation helpers ----
            def pgen(j, pool, width=1024):
                # P columns [j*width, (j+1)*width) in `width/512` matmuls
                pp = pool.tile([C, width], f32, tag=pool.name,
                               name=f"pp{j}_{width}")
                for k in range(width // 512):
                    s = j * width + k * 512
                    nc.tensor.matmul(pp[:, k * 512:(k + 1) * 512], at_t,
                                     img_t[:, s:s + 512],
                                     start=True, stop=True)
                nc.vector.tensor_copy(
                    p_t[:, j * width:(j + 1) * width], pp[:])

            def vgen(t0, cnt, pool, tag=None, on_act=False):
                # [v^T | u] per key tile at 65-col strides in PSUM; eu=exp(u)
                # (ACT); u column then memset to 1.0; per-tile PSUM->SBUF
                # mul by e^u gives vhat = [v*e^u | e^u].
                ps = pool.tile([C, cnt * (Co + 1)], f32,
                               tag=tag or pool.name, name=f"vg{t0}")
                ps3 = ps[:].rearrange("p (t c) -> p t c", c=Co + 1)
                for i in range(cnt):
                    nc.tensor.matmul(
                        ps3[:, i, :],
                        img_t[:, (t0 + i) * 128:(t0 + i + 1) * 128], wvg_t,
                        start=True, stop=True,
                    )
                nc.scalar.activation(eu_t[:, t0:t0 + cnt], ps3[:, :, Co], Exp)
                nc.vector.memset(ps3[:, :, Co], 1.0)
                Copy = mybir.ActivationFunctionType.Copy
                for i in range(cnt):
                    t = t0 + i
                    if on_act:
                        # ACT is idle in the preamble; DVE muls there pay an
                        # EVSEM prelude each and starve the first AV group
                        nc.scalar.activation(vhat_t[:, t, :], ps3[:, i, :],
                                             Copy, scale=eu_t[:, t:t + 1])
                    else:
                        nc.vector.tensor_scalar_mul(
                            vhat_t[:, t, :], ps3[:, i, :], eu_t[:, t:t + 1]
                        )

            # minimal preamble: enough for the exp stream to start
            pgen(0, ps_c3, 512)            # P cols 0:512
            pgen(1, ps_c3, 512)            # P cols 512:1024
            vgen(0, 7, ps_o, tag="po", on_act=True)  # po slot B is free

            # ---- main loop: flat pipeline over 52 (chunk, group) pairs ----
            def emit_av(c, g, po):
                t0, cnt = GRPS[g]
                et = ets[(c * NG + g) % (2 * NG)]
                for i in range(cnt):
                    t = t0 + i
                    nc.tensor.matmul(
                        po[:], vhat_t[:, t, :],
                        et[:, i * 512:(i + 1) * 512],
                        start=(t == 0), stop=(t == MT - 1),
                    )

            # deferred generation work, one item per chunk-0 group
            deferred = {
                0: lambda: pgen(2, ps_x, 512),   # P cols 1024:1536
                1: lambda: vgen(7, 7, ps_x),
                2: lambda: pgen(3, ps_x, 512),
                3: lambda: pgen(4, ps_x, 512),
                4: lambda: pgen(5, ps_x, 512),
                5: lambda: vgen(14, 7, ps_x),
                6: lambda: pgen(6, ps_x, 512),
                7: lambda: pgen(7, ps_x, 512),   # P cols 3584:4096
                8: lambda: vgen(21, 7, ps_x),
                9: lambda: vgen(28, 4, ps_x),
            }

            # postamble for chunk c, spread one op per group via `sched`
            def sched_postamble(sched, c, Gbase):
                st = {}

                def _recip(pin):
                    recip = wpool.tile([1, 512], f32, tag="recip",
                                       name=f"recip{c}")
                    nc.vector.reciprocal(recip[:], pos[c][Co:Co + 1, :])
                    # bf16 copy: the broadcast matmul then runs 1-pass
                    # (fp32 matmuls are 2-pass and stall the PE stream)
                    st["recip"] = wpool.tile([1, 512], bf16, tag="recipb",
                                             name=f"recipb{c}")
                    nc.vector.tensor_copy(st["recip"][:], recip[:])

                def _po2(pin):
                    st["po2"] = ps_x.tile([Co, 512], f32, tag="ps_x",
                                          name=f"po2_{c}")
                    mm = nc.tensor.matmul(st["po2"][:], ones_t[:],
                                          st["recip"][:],
                                          start=True, stop=True)
                    if pin is not None:
                        tile.add_dep_helper(
                            mm.ins, pin.ins, sync=False,
                            reason="postamble bcast after pc stream")

                def _onorm(pin):
                    rb = wpool.tile([Co, 512], f32, tag="rb", name=f"rb{c}")
                    nc.vector.tensor_copy(rb[:], st["po2"][:])
                    st["onorm"] = wpool.tile([Co, 512], bf16, tag="onorm",
                                             name=f"onorm{c}")
                    nc.vector.tensor_mul(st["onorm"][:], pos[c][0:Co, :], rb[:])

                def _py(pin):
                    st["py"] = ps_x.tile([C, 512], f32, tag="ps_x",
                                         name=f"py{c}")
                    mm = nc.tensor.matmul(st["py"][:], wbf_t[:],
                                          st["onorm"][:],
                                          start=True, stop=True)
                    if pin is not None:
                        tile.add_dep_helper(
                            mm.ins, pin.ins, sync=False,
                            reason="postamble proj after pc stream")

                def _fin(pin):
                    yr = wpool.tile([C, 512], f32, tag="yr", name=f"yr{c}")
                    nc.vector.tensor_scalar(yr[:], st["py"][:], bcc_t, 0.0,
                                            Add, Max)
                    nc.vector.tensor_add(
                        fin_t[:, c * 512:(c + 1) * 512], yr[:],
                        img_t[:, c * 512:(c + 1) * 512],
                    )
                    nc.sync.dma_start(out_d[:, c * 512:(c + 1) * 512],
                                      fin_t[:, c * 512:(c + 1) * 512])

                for off, fn in ((0, _recip), (4, _po2), (5, _onorm),
                                (6, _py), (7, _fin)):
                    sched.setdefault(Gbase + off, []).append(fn)

            ets = [None] * (2 * NG)
            pos = [None] * NCH
            NTOT = NCH * NG
            sched = {}
            for G in range(NTOT):
                c, g = divmod(G, NG)
                if g == 0:
                    pos[c] = ps_o.tile([Co + 1, 512], f32, tag="po",
                                       name=f"po{c}")
                t0, cnt = GRPS[g]
                pool = ps_c3 if cnt == 3 else ps_c2
                pc = pool.tile([128, cnt * 512], f32, tag=pool.name,
                               name=f"pc{G}")
                for i in range(cnt):
                    t = t0 + i
                    cur_pc_mm = nc.tensor.matmul(
                        pc[:, i * 512:(i + 1) * 512],
                        p_t[:, t * 128:(t + 1) * 128],
                        rng_t[:, c * 512:(c + 1) * 512],
                        start=True, stop=True,
                    )
                et = epool.tile([128, cnt * 512], bf16, tag="et", name="et")
                nc.scalar.activation(et, pc, Exp)
                ets[G % (2 * NG)] = et
                J = G - LAG
                if J >= 0:
                    jc, jg = divmod(J, NG)
                    emit_av(jc, jg, pos[jc])
                    if jg == NG - 1 and jc < NCH - 1:
                        sched_postamble(sched, jc, G + 1)
                if c == 0 and g in deferred:
                    deferred[g]()
                for fn in sched.pop(G, ()):
                    fn(cur_pc_mm)
            for J in range(NTOT - LAG, NTOT):
                jc, jg = divmod(J, NG)
                emit_av(jc, jg, pos[jc])
            for Gv in sorted(sched):
                for fn in sched.pop(Gv, ()):
                    fn(None)

            # ---- tail: final postamble as two pipelined 256-col halves ----
            c = NCH - 1
            po = pos[c]
            trecs = []
            for h in range(2):
                s = slice(h * 256, (h + 1) * 256)
                recip = wpool.tile([1, 256], f32, tag="recip",
                                    name=f"trec{h}")
                nc.vector.reciprocal(recip[:], po[Co:Co + 1, s])
                recipb = wpool.tile([1, 256], bf16, tag="recipb",
                                    name=f"trecb{h}")
                nc.vector.tensor_copy(recipb[:], recip[:])
                trecs.append(recipb)
            for h in range(2):
                s = slice(h * 256, (h + 1) * 256)
                po2 = ps_x.tile([Co, 256], f32, tag="ps_x", name=f"tpo2_{h}")
                nc.tensor.matmul(po2[:], ones_t[:], trecs[h][:],
                                 start=True, stop=True)
                rb = wpool.tile([Co, 256], f32, tag="rb", name=f"trb{h}")
                nc.vector.tensor_copy(rb[:], po2[:])
                onorm = wpool.tile([Co, 256], bf16, tag="onorm",
                                   name=f"tonorm{h}")
                nc.vector.tensor_mul(onorm[:], po[0:Co, s], rb[:])
                py = ps_o.tile([C, 256], f32, tag="po", name=f"tpy{h}")
                nc.tensor.matmul(py[:], wbf_t[:], onorm[:],
                                 start=True, stop=True)
                yr = wpool.tile([C, 256], f32, tag=f"yr{h}", name=f"tyr{h}")
                nc.scalar.activation(yr[:], py[:], Relu, bias=bcc_t, scale=1.0)
                nc.vector.tensor_add(
                    fin_t[:, c * 512 + h * 256:c * 512 + (h + 1) * 256],
                    yr[:], img_t[:, c * 512 + h * 256:c * 512 + (h + 1) * 256],
                )
                dq = nc.sync if h == 0 else nc.scalar
                dq.dma_start(
                    out_d[:, c * 512 + h * 256:c * 512 + (h + 1) * 256],
                    fin_t[:, c * 512 + h * 256:c * 512 + (h + 1) * 256],
                )

    nc.finalize()
    return nc


def _prepare(range_x, img, wq, bq, wk, bk, wv, bv, wc, bc,
             bn_gamma, bn_beta, bn_mean, bn_var):
    """Build (or fetch) the Bass program and the 8 per-core input maps."""
    import sys
    if "/opt/trn_rl_repo" not in sys.path:
        sys.path.insert(0, "/opt/trn_rl_repo")
    import ml_dtypes

    range_x = np.asarray(range_x, np.float32)
    img = np.asarray(img, np.float32)
    wq = np.asarray(wq, np.float32)
    bq = np.asarray(bq, np.float32)
    wk = np.asarray(wk, np.float32)
    wv = np.asarray(wv, np.float32)
    bv = np.asarray(bv, np.float32)
    wc = np.asarray(wc, np.float32)
    bc = np.asarray(bc, np.float32)
    bn_gamma = np.asarray(bn_gamma, np.float32)
    bn_beta = np.asarray(bn_beta, np.float32)
    bn_mean = np.asarray(bn_mean, np.float32)
    bn_var = np.asarray(bn_var, np.float32)

    B, C, W, H = range_x.shape
    N = W * H
    NQ = N // 2
    Co = wq.shape[0]

    # Host-side weight folding (all tiny).
    inv = bn_gamma / np.sqrt(bn_var + BN_EPS)
    wcp = inv[:, None] * wc                                   # [C, Co]
    bcc = inv * bc + bn_beta - bn_mean * inv + wcp @ bv       # [C]
    at = wk.T @ wq                                            # lhsT for P-gen
    wvg = np.concatenate([wv.T, (wk.T @ bq)[:, None]], axis=1)  # [C, Co+1]

    w16 = np.concatenate([at, wvg], axis=1).astype(np.float16)
    wbf = wcp.T.astype(ml_dtypes.bfloat16)                    # [Co, C]
    x32 = bcc[:, None].astype(np.float32)

    key = (C, N, NQ, Co)
    if key not in _CACHE:
        _CACHE[key] = _build_program(C, N, NQ, Co)
    nc = _CACHE[key]

    n_cores = 8
    in_maps = []
    for core in range(n_cores):
        b, h = core // 2, core % 2
        im = img[b].reshape(C, N)
        # rotate keys so the query half is at columns 0:NQ (softmax is
        # permutation-invariant over keys; residual slice becomes fixed)
        rot = np.concatenate([im[:, h * NQ:], im[:, :h * NQ]], axis=1)
        in_maps.append({
            "w16": w16,
            "img16": rot.astype(np.float16),
            "rng16": range_x[b].reshape(C, N)[:, h * NQ:(h + 1) * NQ]
                     .astype(np.float16),
            "wbf": wbf,
            "x32": x32,
        })

    return nc, in_maps, (B, C, W, H, N, NQ)


def kernel(range_x, img, wq, bq, wk, bk, wv, bv, wc, bc,
           bn_gamma, bn_beta, bn_mean, bn_var):
    import sys
    if "/opt/trn_rl_repo" not in sys.path:
        sys.path.insert(0, "/opt/trn_rl_repo")
    from concourse.bass_utils import run_bass_kernel_spmd

    nc, in_maps, (B, C, W, H, N, NQ) = _prepare(
        range_x, img, wq, bq, wk, bk, wv, bv, wc, bc,
        bn_gamma, bn_beta, bn_mean, bn_var)

    global _LAST_RESULTS
    _LAST_RESULTS = run_bass_kernel_spmd(nc, in_maps, list(range(8)))
    res = _LAST_RESULTS.results

    out = np.empty((B, C, N), np.float32)
    for core in range(8):
        b, h = core // 2, core % 2
        out[b, :, h * NQ:(h + 1) * NQ] = res[core]["out"]
    return out.reshape(B, C, W, H)


# revision 34
# speedup vs baseline: 1.0112x; 1.0112x over previous
"""Trainium2 Bass kernel for the nn_Attention problem (non-local attention).

Reference (per batch b, N = W*H positions):
    q = wq@r + bq; k = wk@i + bk; corr[n,m] = q_n . k_m
    attn = softmax_m(corr); v = wv@i + bv
    out = v @ attn^T; y = relu(BN(wc@out + bc)); result = img + y

Algebraic scheme (P-scheme, K=128 contraction — keeps the PE HAM-warm;
a K=64 direct q^T k variant measured cold/half-clock):
    corr[n,m] = r_n^T A i_m + u[m] + row-const,  A = wq^T wk
  - row-consts cancel in softmax (dropped); u[m] = (wk^T bq) . i_m is
    folded into vhat as a per-key e^u scale.
  - P = A @ i on device ([C,N] fp16); logits tile = P_tile^T @ r.
  - max-subtraction skipped: |logit| < ~55 << 88 (bf16 exp range).
  - Vhat = [v * e^u ; e^u]; po = Vhat^T @ exp(logits); row 64 = softmax
    denominator. bv and BN fold into the output projection/bias on host.
  - Residual: img columns are ROTATED per-core on host so the query half
    always sits at columns 0:NQ (attention is permutation-invariant over
    keys) -> one program for all 8 cores, no separate fp32 residual input.

Precision: logits path fp16, attention-value path bf16, normalization fp32.
Sharding: 8 cores = 4 batches x 2 query halves; full keys per core.

Perf structure (ACT exp stream is the bottleneck: 52 exps, ~69us):
  - input DMA split across SP + ACT hwdge queues and the Pool swdge queue,
    row-split transfers ordered so P-gen starts right after engine bringup
    (PE never idles a HAM window); output DMA overlaps per chunk.
  - dummy exp preloads the ACT exp table during the DMA wait; PE warmup
    matmuls bridge the DMA wait to keep the HAM clock gate open.
  - wide exps amortize ACT per-instr overhead. PSUM: asymmetric pc pools
    (1536-wide x1 + 1024-wide x1, alternating groups) + po x2 + scratch
    = 8 banks; double-buffered po removes every chunk-boundary hazard.
  - FLAT software pipeline over all 52 (chunk, group) pairs: pc/exp lead,
    AV matmuls lag 3 groups, postamble ops spread one-per-group so no PE
    instruction ever waits on the 3.3us DVE reciprocal.
  - second-half P/vhat generation is deferred into chunk 0's groups via
    the scratch bank, so the exp stream starts ~6us after bringup.
  - tail: final postamble runs as two pipelined 256-column halves.
"""

import numpy as np

BN_EPS = 1e-5

_CACHE: dict = {}
_LAST_RESULTS = None  # BassKernelResults of the most recent run (for profiling)


def _build_program(C: int, N: int, NQ: int, Co: int):
    import concourse.bass as bass
    import concourse.tile as tile
    from concourse import bacc, mybir

    f32 = mybir.dt.float32
    f16 = mybir.dt.float16
    bf16 = mybir.dt.bfloat16
    Exp = mybir.ActivationFunctionType.Exp
    Relu = mybir.ActivationFunctionType.Relu
    Add = mybir.AluOpType.add
    Max = mybir.AluOpType.max

    MT = N // 128        # key tiles (32)
    NCH = NQ // 512      # query chunks per core (4)
    # groups alternate 3/2 key tiles (1536/1024-wide exps): 13 per chunk
    GRPS = []
    t = 0
    while t < MT:
        cnt = 3 if (len(GRPS) % 2 == 0 and t + 3 <= MT) else min(2, MT - t)
        GRPS.append((t, cnt))
        t += cnt
    NG = len(GRPS)       # 13
    LAG = 3

    nc = bacc.Bacc()
    w16_d = nc.declare_dram_parameter("w16", [C, C + Co + 1], f16, isOutput=False)
    img_d = nc.declare_dram_parameter("img16", [C, N], f16, isOutput=False)
    rng_d = nc.declare_dram_parameter("rng16", [C, NQ], f16, isOutput=False)
    wbf_d = nc.declare_dram_parameter("wbf", [Co, C], bf16, isOutput=False)
    x32_d = nc.declare_dram_parameter("x32", [C, 1], f32, isOutput=False)
    out_d = nc.declare_dram_parameter("out", [C, NQ], f32, isOutput=True)

    with tile.TileContext(nc) as tc:
        with (
            tc.tile_pool(name="const", bufs=1) as cpool,
            tc.tile_pool(name="work", bufs=4) as wpool,
            tc.tile_pool(name="et", bufs=2 * NG) as epool,
            tc.tile_pool(name="ps_c3", bufs=1, space="PSUM") as ps_c3,
            tc.tile_pool(name="ps_c2", bufs=1, space="PSUM") as ps_c2,
            tc.tile_pool(name="ps_o", bufs=2, space="PSUM") as ps_o,
            tc.tile_pool(name="ps_x", bufs=1, space="PSUM") as ps_x,
        ):
            img_t = cpool.tile([C, N], f16)
            rng_t = cpool.tile([C, NQ], f16)
            w16_t = cpool.tile([C, C + Co + 1], f16)
            wbf_t = cpool.tile([Co, C], bf16)
            x32_t = cpool.tile([C, 1], f32)
            at_t = w16_t[:, 0:C]                 # A^T = wk^T wq
            wvg_t = w16_t[:, C:C + Co + 1]       # [wv^T | wk^T bq]
            bcc_t = x32_t[:, 0:1]

            # ---- input DMAs: SP + ACT hwdge queues + Pool swdge ----
            # img q1 leads on both hw queues (gates P-gen); tiny rng chunk 0
            # second (gates the first exp); weights ride the Pool swdge.
            H2 = C // 2
            nc.sync.dma_start(w16_t[:], w16_d[:])
            nc.scalar.dma_start(img_t[H2:C, 0:1024], img_d[H2:C, 0:1024])
            nc.sync.dma_start(img_t[0:H2, 0:1024], img_d[0:H2, 0:1024])
            nc.scalar.dma_start(rng_t[H2:C, 0:512], rng_d[H2:C, 0:512])
            nc.sync.dma_start(rng_t[0:H2, 0:512], rng_d[0:H2, 0:512])
            nc.gpsimd.dma_start(x32_t[:], x32_d[:])
            nc.sync.dma_start(img_t[0:H2, 1024:2048], img_d[0:H2, 1024:2048])
            nc.scalar.dma_start(img_t[H2:C, 1024:2048], img_d[H2:C, 1024:2048])
            nc.gpsimd.dma_start(img_t[0:H2, 2048:3072], img_d[0:H2, 2048:3072])
            nc.gpsimd.dma_start(img_t[H2:C, 2048:3072], img_d[H2:C, 2048:3072])
            nc.sync.dma_start(rng_t[0:H2, 512:NQ], rng_d[0:H2, 512:NQ])
            nc.scalar.dma_start(rng_t[H2:C, 512:NQ], rng_d[H2:C, 512:NQ])
            nc.gpsimd.dma_start(img_t[0:H2, 3072:N], img_d[0:H2, 3072:N])
            nc.gpsimd.dma_start(img_t[H2:C, 3072:N], img_d[H2:C, 3072:N])
            nc.scalar.dma_start(wbf_t[:], wbf_d[:])

            p_t = cpool.tile([C, N], f16)            # P = A @ img
            vhat_t = cpool.tile([128, MT, Co + 1], bf16)
            eu_t = cpool.tile([128, MT], f32)        # e^u per key
            fin_t = cpool.tile([C, NQ], f32)
            ones_t = cpool.tile([1, Co], bf16)
            dummy_t = cpool.tile([1, 2], f32)
            warm_t = cpool.tile([C, 512], f16)

            nc.vector.memset(ones_t[:], 1.0)
            nc.vector.memset(dummy_t[:], 0.0)
            nc.vector.memset(warm_t[:], 0.0)
            # preload the exp table while DMAs run
            nc.scalar.activation(dummy_t[0:1, 1:2], dummy_t[0:1, 0:1], Exp)

            # PE warmup during the DMA wait (HAM needs ~3.4us of activity)
            warm_ps = ps_o.tile([Co + 1, 512], f32, tag="po")
            for _ in range(7):
                nc.tensor.matmul(warm_ps[:], warm_t[:, 0:Co + 1],
                                 warm_t[:], start=True, stop=True)

            # ---- P / vhat generation helpers ----
            def pgen(j, pool, width=1024):
                # P columns [j*width, (j+1)*width) in `width/512` matmuls
                pp = pool.tile([C, width], f32, tag=pool.name,
                               name=f"pp{j}_{width}")
                for k in range(width // 512):
                    s = j * width + k * 512
                    nc.tensor.matmul(pp[:, k * 512:(k + 1) * 512], at_t,
                                     img_t[:, s:s + 512],
                                     start=True, stop=True)
                nc.vector.tensor_copy(
                    p_t[:, j * width:(j + 1) * width], pp[:])

            def vgen(t0, cnt, pool, tag=None, on_act=False):
                # [v^T | u] per key tile at 65-col strides in PSUM; eu=exp(u)
                # (ACT); u column then memset to 1.0; per-tile PSUM->SBUF
                # mul by e^u gives vhat = [v*e^u | e^u].
                ps = pool.tile([C, cnt * (Co + 1)], f32,
                               tag=tag or pool.name, name=f"vg{t0}")
                ps3 = ps[:].rearrange("p (t c) -> p t c", c=Co + 1)
                for i in range(cnt):
                    nc.tensor.matmul(
                        ps3[:, i, :],
                        img_t[:, (t0 + i) * 128:(t0 + i + 1) * 128], wvg_t,
                        start=True, stop=True,
                    )
                nc.scalar.activation(eu_t[:, t0:t0 + cnt], ps3[:, :, Co], Exp)
                nc.vector.memset(ps3[:, :, Co], 1.0)
                Copy = mybir.ActivationFunctionType.Copy
                for i in range(cnt):
                    t = t0 + i
                    if on_act:
                        # ACT is idle in the preamble; DVE muls there pay an
                        # EVSEM prelude each and starve the first AV group
                        nc.scalar.activation(vhat_t[:, t, :], ps3[:, i, :],
                                             Copy, scale=eu_t[:, t:t + 1])
                    else:
                        nc.vector.tensor_scalar_mul(
                            vhat_t[:, t, :], ps3[:, i, :], eu_t[:, t:t + 1]
                        )

            # minimal preamble: enough for the exp stream to start
            pgen(0, ps_c3, 512)            # P cols 0:512
            pgen(1, ps_c3, 512)            # P cols 512:1024
            vgen(0, 7, ps_o, tag="po", on_act=True)  # po slot B is free

            # ---- main loop: flat pipeline over 52 (chunk, group) pairs ----
            def emit_av(c, g, po):
                t0, cnt = GRPS[g]
                et = ets[(c * NG + g) % (2 * NG)]
                for i in range(cnt):
                    t = t0 + i
                    nc.tensor.matmul(
                        po[:], vhat_t[:, t, :],
                        et[:, i * 512:(i + 1) * 512],
                        start=(t == 0), stop=(t == MT - 1),
                    )

            # deferred generation work, one item per chunk-0 group
            deferred = {
                0: lambda: pgen(2, ps_x, 512),   # P cols 1024:1536
                1: lambda: vgen(7, 7, ps_x),
                2: lambda: pgen(3, ps_x, 512),
                3: lambda: pgen(4, ps_x, 512),
                4: lambda: pgen(5, ps_x, 512),
                5: lambda: vgen(14, 7, ps_x),
                6: lambda: pgen(6, ps_x, 512),
                7: lambda: pgen(7, ps_x, 512),   # P cols 3584:4096
                8: lambda: vgen(21, 7, ps_x),
                9: lambda: vgen(28, 4, ps_x),
            }

            # postamble for chunk c, spread one op per group via `sched`
            def sched_postamble(sched, c, Gbase):
                st = {}

                def _recip(pin):
                    recip = wpool.tile([1, 512], f32, tag="recip",
                                       name=f"recip{c}")
                    nc.vector.reciprocal(recip[:], pos[c][Co:Co + 1, :])
                    # bf16 copy: the broadcast matmul then runs 1-pass
                    # (fp32 matmuls are 2-pass and stall the PE stream)
                    st["recip"] = wpool.tile([1, 512], bf16, tag="recipb",
                                             name=f"recipb{c}")
                    nc.vector.tensor_copy(st["recip"][:], recip[:])

                def _po2(pin):
                    st["po2"] = ps_x.tile([Co, 512], f32, tag="ps_x",
                                          name=f"po2_{c}")
                    mm = nc.tensor.matmul(st["po2"][:], ones_t[:],
                                          st["recip"][:],
                                          start=True, stop=True)
                    if pin is not None:
                        tile.add_dep_helper(
                            mm.ins, pin.ins, sync=False,
                            reason="postamble bcast after pc stream")

                def _onorm(pin):
                    rb = wpool.tile([Co, 512], f32, tag="rb", name=f"rb{c}")
                    nc.vector.tensor_copy(rb[:], st["po2"][:])
                    st["onorm"] = wpool.tile([Co, 512], bf16, tag="onorm",
                                             name=f"onorm{c}")
                    nc.vector.tensor_mul(st["onorm"][:], pos[c][0:Co, :], rb[:])

                def _py(pin):
                    st["py"] = ps_x.tile([C, 512], f32, tag="ps_x",
                                         name=f"py{c}")
                    mm = nc.tensor.matmul(st["py"][:], wbf_t[:],
                                          st["onorm"][:],
                                          start=True, stop=True)
                    if pin is not None:
                        tile.add_dep_helper(
                            mm.ins, pin.ins, sync=False,
                            reason="postamble proj after pc stream")

                def _fin(pin):
                    yr = wpool.tile([C, 512], f32, tag="yr", name=f"yr{c}")
                    nc.vector.tensor_scalar(yr[:], st["py"][:], bcc_t, 0.0,
                                            Add, Max)
                    nc.vector.tensor_add(
                        fin_t[:, c * 512:(c + 1) * 512], yr[:],
                        img_t[:, c * 512:(c + 1) * 512],
                    )
                    nc.sync.dma_start(out_d[:, c * 512:(c + 1) * 512],
                                      fin_t[:, c * 512:(c + 1) * 512])

                for off, fn in ((0, _recip), (4, _po2), (5, _onorm),
                                (6, _py), (7, _fin)):
                    sched.setdefault(Gbase + off, []).append(fn)

            ets = [None] * (2 * NG)
            pos = [None] * NCH
            NTOT = NCH * NG
            sched = {}
            for G in range(NTOT):
                c, g = divmod(G, NG)
                if g == 0:
                    pos[c] = ps_o.tile([Co + 1, 512], f32, tag="po",
                                       name=f"po{c}")
                t0, cnt = GRPS[g]
                pool = ps_c3 if cnt == 3 else ps_c2
                pc = pool.tile([128, cnt * 512], f32, tag=pool.name,
                               name=f"pc{G}")
                for i in range(cnt):
                    t = t0 + i
                    cur_pc_mm = nc.tensor.matmul(
                        pc[:, i * 512:(i + 1) * 512],
                        p_t[:, t * 128:(t + 1) * 128],
                        rng_t[:, c * 512:(c + 1) * 512],
                        start=True, stop=True,
                    )
                et = epool.tile([128, cnt * 512], bf16, tag="et", name="et")
                nc.scalar.activation(et, pc, Exp)
                ets[G % (2 * NG)] = et
                J = G - LAG
                if J >= 0:
                    jc, jg = divmod(J, NG)
                    emit_av(jc, jg, pos[jc])
                    if jg == NG - 1 and jc < NCH - 1:
                        sched_postamble(sched, jc, G + 1)
                if c == 0 and g in deferred:
                    deferred[g]()
                for fn in sched.pop(G, ()):
                    fn(cur_pc_mm)
            for J in range(NTOT - LAG, NTOT):
                jc, jg = divmod(J, NG)
                emit_av(jc, jg, pos[jc])
            for Gv in sorted(sched):
                for fn in sched.pop(Gv, ()):
                    fn(None)

            # ---- tail: final postamble as two pipelined 256-col halves ----
            c = NCH - 1
            po = pos[c]
            trecs = []
            for h in range(2):
                s = slice(h * 256, (h + 1) * 256)
                recip = wpool.tile([1, 256], f32, tag="recip",
                                    name=f"trec{h}")
                nc.vector.reciprocal(recip[:], po[Co:Co + 1, s])
                recipb = wpool.tile([1, 256], bf16, tag="recipb",
                                    name=f"trecb{h}")
                nc.vector.tensor_copy(recipb[:], recip[:])
                trecs.append(recipb)
            for h in range(2):
                s = slice(h * 256, (h + 1) * 256)
                po2 = ps_x.tile([Co, 256], f32, tag="ps_x", name=f"tpo2_{h}")
                nc.tensor.matmul(po2[:], ones_t[:], trecs[h][:],
                                 start=True, stop=True)
                rb = wpool.tile([Co, 256], f32, tag="rb", name=f"trb{h}")
                nc.vector.tensor_copy(rb[:], po2[:])
                onorm = wpool.tile([Co, 256], bf16, tag="onorm",
                                   name=f"tonorm{h}")
                nc.vector.tensor_mul(onorm[:], po[0:Co, s], rb[:])
                py = ps_o.tile([C, 256], f32, tag="po", name=f"tpy{h}")
                nc.tensor.matmul(py[:], wbf_t[:], onorm[:],
                                 start=True, stop=True)
                yr = wpool.tile([C, 256], f32, tag=f"yr{h}", name=f"tyr{h}")
                nc.scalar.activation(yr[:], py[:], Relu, bias=bcc_t, scale=1.0)
                nc.vector.tensor_add(
                    fin_t[:, c * 512 + h * 256:c * 512 + (h + 1) * 256],
                    yr[:], img_t[:, c * 512 + h * 256:c * 512 + (h + 1) * 256],
                )
                dq = nc.sync if h == 0 else nc.scalar
                dq.dma_start(
                    out_d[:, c * 512 + h * 256:c * 512 + (h + 1) * 256],
                    fin_t[:, c * 512 + h * 256:c * 512 + (h + 1) * 256],
                )

    nc.finalize()
    return nc


def _prepare(range_x, img, wq, bq, wk, bk, wv, bv, wc, bc,
             bn_gamma, bn_beta, bn_mean, bn_var):
    """Build (or fetch) the Bass program and the 8 per-core input maps."""
    import sys
    if "/opt/trn_rl_repo" not in sys.path:
        sys.path.insert(0, "/opt/trn_rl_repo")
    import ml_dtypes

    range_x = np.asarray(range_x, np.float32)
    img = np.asarray(img, np.float32)
    wq = np.asarray(wq, np.float32)
    bq = np.asarray(bq, np.float32)
    wk = np.asarray(wk, np.float32)
    wv = np.asarray(wv, np.float32)
    bv = np.asarray(bv, np.float32)
    wc = np.asarray(wc, np.float32)
    bc = np.asarray(bc, np.float32)
    bn_gamma = np.asarray(bn_gamma, np.float32)
    bn_beta = np.asarray(bn_beta, np.float32)
    bn_mean = np.asarray(bn_mean, np.float32)
    bn_var = np.asarray(bn_var, np.float32)

    B, C, W, H = range_x.shape
    N = W * H
    NQ = N // 2
    Co = wq.shape[0]

    # Host-side weight folding (all tiny).
    inv = bn_gamma / np.sqrt(bn_var + BN_EPS)
    wcp = inv[:, None] * wc                                   # [C, Co]
    bcc = inv * bc + bn_beta - bn_mean * inv + wcp @ bv       # [C]
    at = wk.T @ wq                                            # lhsT for P-gen
    wvg = np.concatenate([wv.T, (wk.T @ bq)[:, None]], axis=1)  # [C, Co+1]

    w16 = np.concatenate([at, wvg], axis=1).astype(np.float16)
    wbf = wcp.T.astype(ml_dtypes.bfloat16)                    # [Co, C]
    x32 = bcc[:, None].astype(np.float32)

    key = (C, N, NQ, Co)
    if key not in _CACHE:
        _CACHE[key] = _build_program(C, N, NQ, Co)
    nc = _CACHE[key]

    n_cores = 8
    in_maps = []
    for core in range(n_cores):
        b, h = core // 2, core % 2
        im = img[b].reshape(C, N)
        # rotate keys so the query half is at columns 0:NQ (softmax is
        # permutation-invariant over keys; residual slice becomes fixed)
        rot = np.concatenate([im[:, h * NQ:], im[:, :h * NQ]], axis=1)
        in_maps.append({
            "w16": w16,
            "img16": rot.astype(np.float16),
            "rng16": range_x[b].reshape(C, N)[:, h * NQ:(h + 1) * NQ]
                     .astype(np.float16),
            "wbf": wbf,
            "x32": x32,
        })

    return nc, in_maps, (B, C, W, H, N, NQ)


def kernel(range_x, img, wq, bq, wk, bk, wv, bv, wc, bc,
           bn_gamma, bn_beta, bn_mean, bn_var):
    import sys
    if "/opt/trn_rl_repo" not in sys.path:
        sys.path.insert(0, "/opt/trn_rl_repo")
    from concourse.bass_utils import run_bass_kernel_spmd

    nc, in_maps, (B, C, W, H, N, NQ) = _prepare(
        range_x, img, wq, bq, wk, bk, wv, bv, wc, bc,
        bn_gamma, bn_beta, bn_mean, bn_var)

    global _LAST_RESULTS
    _LAST_RESULTS = run_bass_kernel_spmd(nc, in_maps, list(range(8)))
    res = _LAST_RESULTS.results

    out = np.empty((B, C, N), np.float32)
    for core in range(8):
        b, h = core // 2, core % 2
        out[b, :, h * NQ:(h + 1) * NQ] = res[core]["out"]
    return out.reshape(B, C, W, H)


# revision 35
# speedup vs baseline: 1.0292x; 1.0178x over previous
"""Trainium2 Bass kernel for the nn_Attention problem (non-local attention).

Reference (per batch b, N = W*H positions):
    q = wq@r + bq; k = wk@i + bk; corr[n,m] = q_n . k_m
    attn = softmax_m(corr); v = wv@i + bv
    out = v @ attn^T; y = relu(BN(wc@out + bc)); result = img + y

Algebraic scheme (P-scheme, K=128 contraction — keeps the PE HAM-warm;
a K=64 direct q^T k variant measured cold/half-clock):
    corr[n,m] = r_n^T A i_m + u[m] + row-const,  A = wq^T wk
  - row-consts cancel in softmax (dropped); u[m] = (wk^T bq) . i_m is
    folded into vhat as a per-key e^u scale.
  - P = A @ i on device ([C,N] fp16); logits tile = P_tile^T @ r.
  - max-subtraction skipped: |logit| < ~55 << 88 (bf16 exp range).
  - Vhat = [v * e^u ; e^u]; po = Vhat^T @ exp(logits); row 64 = softmax
    denominator. bv and BN fold into the output projection/bias on host.
  - Residual: img columns are ROTATED per-core on host so the query half
    always sits at columns 0:NQ (attention is permutation-invariant over
    keys) -> one program for all 8 cores, no separate fp32 residual input.

Precision: logits path fp16, attention-value path bf16, normalization fp32.
Sharding: 8 cores = 4 batches x 2 query halves; full keys per core.

Perf structure (ACT exp stream is the bottleneck: 52 exps, ~69us):
  - input DMA split across SP + ACT hwdge queues and the Pool swdge queue,
    row-split transfers ordered so P-gen starts right after engine bringup
    (PE never idles a HAM window); output DMA overlaps per chunk.
  - dummy exp preloads the ACT exp table during the DMA wait; PE warmup
    matmuls bridge the DMA wait to keep the HAM clock gate open.
  - wide exps amortize ACT per-instr overhead. PSUM: asymmetric pc pools
    (1536-wide x1 + 1024-wide x1, alternating groups) + po x2 + scratch
    = 8 banks; double-buffered po removes every chunk-boundary hazard.
  - FLAT software pipeline over all 52 (chunk, group) pairs: pc/exp lead,
    AV matmuls lag 3 groups, postamble ops spread one-per-group so no PE
    instruction ever waits on the 3.3us DVE reciprocal.
  - second-half P/vhat generation is deferred into chunk 0's groups via
    the scratch bank, so the exp stream starts ~6us after bringup.
  - tail: final postamble runs as two pipelined 256-column halves.
"""

import numpy as np

BN_EPS = 1e-5

_CACHE: dict = {}
_LAST_RESULTS = None  # BassKernelResults of the most recent run (for profiling)


def _build_program(C: int, N: int, NQ: int, Co: int):
    import concourse.bass as bass
    import concourse.tile as tile
    from concourse import bacc, mybir

    f32 = mybir.dt.float32
    f16 = mybir.dt.float16
    bf16 = mybir.dt.bfloat16
    Exp = mybir.ActivationFunctionType.Exp
    Relu = mybir.ActivationFunctionType.Relu
    Add = mybir.AluOpType.add
    Max = mybir.AluOpType.max

    MT = N // 128        # key tiles (32)
    NCH = NQ // 512      # query chunks per core (4)
    # groups alternate 3/2 key tiles (1536/1024-wide exps): 13 per chunk
    GRPS = []
    t = 0
    while t < MT:
        cnt = 3 if (len(GRPS) % 2 == 0 and t + 3 <= MT) else min(2, MT - t)
        GRPS.append((t, cnt))
        t += cnt
    NG = len(GRPS)       # 13
    LAG = 3

    nc = bacc.Bacc()
    w16_d = nc.declare_dram_parameter("w16", [C, C + Co + 1], f16, isOutput=False)
    img_d = nc.declare_dram_parameter("img16", [C, N], f16, isOutput=False)
    rng_d = nc.declare_dram_parameter("rng16", [C, NQ], f16, isOutput=False)
    wbf_d = nc.declare_dram_parameter("wbf", [Co, C], bf16, isOutput=False)
    x32_d = nc.declare_dram_parameter("x32", [C, 1], f32, isOutput=False)
    out_d = nc.declare_dram_parameter("out", [C, NQ], f32, isOutput=True)

    with tile.TileContext(nc) as tc:
        with (
            tc.tile_pool(name="const", bufs=1) as cpool,
            tc.tile_pool(name="work", bufs=4) as wpool,
            tc.tile_pool(name="et", bufs=2 * NG) as epool,
            tc.tile_pool(name="ps_c3", bufs=1, space="PSUM") as ps_c3,
            tc.tile_pool(name="ps_c2", bufs=1, space="PSUM") as ps_c2,
            tc.tile_pool(name="ps_o", bufs=2, space="PSUM") as ps_o,
            tc.tile_pool(name="ps_x", bufs=1, space="PSUM") as ps_x,
        ):
            img_t = cpool.tile([C, N], f16)
            rng_t = cpool.tile([C, NQ], f16)
            w16_t = cpool.tile([C, C + Co + 1], f16)
            wbf_t = cpool.tile([Co, C], bf16)
            x32_t = cpool.tile([C, 1], f32)
            at_t = w16_t[:, 0:C]                 # A^T = wk^T wq
            wvg_t = w16_t[:, C:C + Co + 1]       # [wv^T | wk^T bq]
            bcc_t = x32_t[:, 0:1]

            # ---- input DMAs: SP + ACT hwdge queues + Pool swdge ----
            # img q1 leads on both hw queues (gates P-gen); tiny rng chunk 0
            # second (gates the first exp); weights ride the Pool swdge.
            H2 = C // 2
            nc.sync.dma_start(img_t[0:H2, 0:1024], img_d[0:H2, 0:1024])
            nc.scalar.dma_start(img_t[H2:C, 0:1024], img_d[H2:C, 0:1024])
            nc.sync.dma_start(rng_t[0:H2, 0:512], rng_d[0:H2, 0:512])
            nc.scalar.dma_start(rng_t[H2:C, 0:512], rng_d[H2:C, 0:512])
            nc.gpsimd.dma_start(x32_t[:], x32_d[:])
            nc.gpsimd.dma_start(w16_t[:], w16_d[:])
            nc.sync.dma_start(img_t[0:H2, 1024:2048], img_d[0:H2, 1024:2048])
            nc.scalar.dma_start(img_t[H2:C, 1024:2048], img_d[H2:C, 1024:2048])
            nc.gpsimd.dma_start(img_t[0:H2, 2048:3072], img_d[0:H2, 2048:3072])
            nc.gpsimd.dma_start(img_t[H2:C, 2048:3072], img_d[H2:C, 2048:3072])
            nc.sync.dma_start(rng_t[0:H2, 512:NQ], rng_d[0:H2, 512:NQ])
            nc.scalar.dma_start(rng_t[H2:C, 512:NQ], rng_d[H2:C, 512:NQ])
            nc.gpsimd.dma_start(img_t[0:H2, 3072:N], img_d[0:H2, 3072:N])
            nc.gpsimd.dma_start(img_t[H2:C, 3072:N], img_d[H2:C, 3072:N])
            nc.scalar.dma_start(wbf_t[:], wbf_d[:])

            p_t = cpool.tile([C, N], f16)            # P = A @ img
            vhat_t = cpool.tile([128, MT, Co + 1], bf16)
            eu_t = cpool.tile([128, MT], f32)        # e^u per key
            fin_t = cpool.tile([C, NQ], f32)
            ones_t = cpool.tile([1, Co], bf16)
            dummy_t = cpool.tile([1, 2], f32)
            warm_t = cpool.tile([C, 512], f16)

            nc.vector.memset(ones_t[:], 1.0)
            nc.vector.memset(dummy_t[:], 0.0)
            nc.vector.memset(warm_t[:], 0.0)
            # preload the exp table while DMAs run
            nc.scalar.activation(dummy_t[0:1, 1:2], dummy_t[0:1, 0:1], Exp)

            # PE warmup during the DMA wait (HAM needs ~3.4us of activity)
            warm_ps = ps_o.tile([Co + 1, 512], f32, tag="po")
            for _ in range(7):
                nc.tensor.matmul(warm_ps[:], warm_t[:, 0:Co + 1],
                                 warm_t[:], start=True, stop=True)

            # ---- P / vhat generation helpers ----
            def pgen(j, pool, width=1024):
                # P columns [j*width, (j+1)*width) in `width/512` matmuls
                pp = pool.tile([C, width], f32, tag=pool.name,
                               name=f"pp{j}_{width}")
                for k in range(width // 512):
                    s = j * width + k * 512
                    nc.tensor.matmul(pp[:, k * 512:(k + 1) * 512], at_t,
                                     img_t[:, s:s + 512],
                                     start=True, stop=True)
                nc.vector.tensor_copy(
                    p_t[:, j * width:(j + 1) * width], pp[:])

            def vgen(t0, cnt, pool, tag=None, on_act=False):
                # [v^T | u] per key tile at 65-col strides in PSUM; eu=exp(u)
                # (ACT); u column then memset to 1.0; per-tile PSUM->SBUF
                # mul by e^u gives vhat = [v*e^u | e^u].
                ps = pool.tile([C, cnt * (Co + 1)], f32,
                               tag=tag or pool.name, name=f"vg{t0}")
                ps3 = ps[:].rearrange("p (t c) -> p t c", c=Co + 1)
                for i in range(cnt):
                    nc.tensor.matmul(
                        ps3[:, i, :],
                        img_t[:, (t0 + i) * 128:(t0 + i + 1) * 128], wvg_t,
                        start=True, stop=True,
                    )
                nc.scalar.activation(eu_t[:, t0:t0 + cnt], ps3[:, :, Co], Exp)
                nc.vector.memset(ps3[:, :, Co], 1.0)
                Copy = mybir.ActivationFunctionType.Copy
                for i in range(cnt):
                    t = t0 + i
                    if on_act:
                        # ACT is idle in the preamble; DVE muls there pay an
                        # EVSEM prelude each and starve the first AV group
                        nc.scalar.activation(vhat_t[:, t, :], ps3[:, i, :],
                                             Copy, scale=eu_t[:, t:t + 1])
                    else:
                        nc.vector.tensor_scalar_mul(
                            vhat_t[:, t, :], ps3[:, i, :], eu_t[:, t:t + 1]
                        )

            # minimal preamble: enough for the exp stream to start
            pgen(0, ps_c3, 512)            # P cols 0:512
            pgen(1, ps_c3, 512)            # P cols 512:1024
            vgen(0, 7, ps_o, tag="po", on_act=True)  # po slot B is free

            # ---- main loop: flat pipeline over 52 (chunk, group) pairs ----
            def emit_av(c, g, po):
                t0, cnt = GRPS[g]
                et = ets[(c * NG + g) % (2 * NG)]
                for i in range(cnt):
                    t = t0 + i
                    nc.tensor.matmul(
                        po[:], vhat_t[:, t, :],
                        et[:, i * 512:(i + 1) * 512],
                        start=(t == 0), stop=(t == MT - 1),
                    )

            # deferred generation work, one item per chunk-0 group
            deferred = {
                0: lambda: pgen(2, ps_x, 512),   # P cols 1024:1536
                1: lambda: vgen(7, 7, ps_x),
                2: lambda: pgen(3, ps_x, 512),
                3: lambda: pgen(4, ps_x, 512),
                4: lambda: pgen(5, ps_x, 512),
                5: lambda: vgen(14, 7, ps_x),
                6: lambda: pgen(6, ps_x, 512),
                7: lambda: pgen(7, ps_x, 512),   # P cols 3584:4096
                8: lambda: vgen(21, 7, ps_x),
                9: lambda: vgen(28, 4, ps_x),
            }

            # postamble for chunk c, spread one op per group via `sched`
            def sched_postamble(sched, c, Gbase):
                st = {}

                def _recip(pin):
                    recip = wpool.tile([1, 512], f32, tag="recip",
                                       name=f"recip{c}")
                    nc.vector.reciprocal(recip[:], pos[c][Co:Co + 1, :])
                    # bf16 copy: the broadcast matmul then runs 1-pass
                    # (fp32 matmuls are 2-pass and stall the PE stream)
                    st["recip"] = wpool.tile([1, 512], bf16, tag="recipb",
                                             name=f"recipb{c}")
                    nc.vector.tensor_copy(st["recip"][:], recip[:])

                def _po2(pin):
                    st["po2"] = ps_x.tile([Co, 512], f32, tag="ps_x",
                                          name=f"po2_{c}")
                    mm = nc.tensor.matmul(st["po2"][:], ones_t[:],
                                          st["recip"][:],
                                          start=True, stop=True)
                    if pin is not None:
                        tile.add_dep_helper(
                            mm.ins, pin.ins, sync=False,
                            reason="postamble bcast after pc stream")

                def _onorm(pin):
                    rb = wpool.tile([Co, 512], f32, tag="rb", name=f"rb{c}")
                    nc.vector.tensor_copy(rb[:], st["po2"][:])
                    st["onorm"] = wpool.tile([Co, 512], bf16, tag="onorm",
                                             name=f"onorm{c}")
                    nc.vector.tensor_mul(st["onorm"][:], pos[c][0:Co, :], rb[:])

                def _py(pin):
                    st["py"] = ps_x.tile([C, 512], f32, tag="ps_x",
                                         name=f"py{c}")
                    mm = nc.tensor.matmul(st["py"][:], wbf_t[:],
                                          st["onorm"][:],
                                          start=True, stop=True)
                    if pin is not None:
                        tile.add_dep_helper(
                            mm.ins, pin.ins, sync=False,
                            reason="postamble proj after pc stream")

                def _fin(pin):
                    yr = wpool.tile([C, 512], f32, tag="yr", name=f"yr{c}")
                    nc.vector.tensor_scalar(yr[:], st["py"][:], bcc_t, 0.0,
                                            Add, Max)
                    nc.vector.tensor_add(
                        fin_t[:, c * 512:(c + 1) * 512], yr[:],
                        img_t[:, c * 512:(c + 1) * 512],
                    )
                    nc.sync.dma_start(out_d[:, c * 512:(c + 1) * 512],
                                      fin_t[:, c * 512:(c + 1) * 512])

                for off, fn in ((0, _recip), (4, _po2), (5, _onorm),
                                (6, _py), (7, _fin)):
                    sched.setdefault(Gbase + off, []).append(fn)

            ets = [None] * (2 * NG)
            pos = [None] * NCH
            NTOT = NCH * NG
            sched = {}
            for G in range(NTOT):
                c, g = divmod(G, NG)
                if g == 0:
                    pos[c] = ps_o.tile([Co + 1, 512], f32, tag="po",
                                       name=f"po{c}")
                t0, cnt = GRPS[g]
                pool = ps_c3 if cnt == 3 else ps_c2
                pc = pool.tile([128, cnt * 512], f32, tag=pool.name,
                               name=f"pc{G}")
                for i in range(cnt):
                    t = t0 + i
                    cur_pc_mm = nc.tensor.matmul(
                        pc[:, i * 512:(i + 1) * 512],
                        p_t[:, t * 128:(t + 1) * 128],
                        rng_t[:, c * 512:(c + 1) * 512],
                        start=True, stop=True,
                    )
                et = epool.tile([128, cnt * 512], bf16, tag="et", name="et")
                nc.scalar.activation(et, pc, Exp)
                ets[G % (2 * NG)] = et
                J = G - LAG
                if J >= 0:
                    jc, jg = divmod(J, NG)
                    emit_av(jc, jg, pos[jc])
                    if jg == NG - 1 and jc < NCH - 1:
                        sched_postamble(sched, jc, G + 1)
                if c == 0 and g in deferred:
                    deferred[g]()
                for fn in sched.pop(G, ()):
                    fn(cur_pc_mm)
            for J in range(NTOT - LAG, NTOT):
                jc, jg = divmod(J, NG)
                emit_av(jc, jg, pos[jc])
            for Gv in sorted(sched):
                for fn in sched.pop(Gv, ()):
                    fn(None)

            # ---- tail: final postamble as two pipelined 256-col halves ----
            c = NCH - 1
            po = pos[c]
            trecs = []
            for h in range(2):
                s = slice(h * 256, (h + 1) * 256)
                recip = wpool.tile([1, 256], f32, tag="recip",
                                    name=f"trec{h}")
                nc.vector.reciprocal(recip[:], po[Co:Co + 1, s])
                recipb = wpool.tile([1, 256], bf16, tag="recipb",
                                    name=f"trecb{h}")
                nc.vector.tensor_copy(recipb[:], recip[:])
                trecs.append(recipb)
            for h in range(2):
                s = slice(h * 256, (h + 1) * 256)
                po2 = ps_x.tile([Co, 256], f32, tag="ps_x", name=f"tpo2_{h}")
                nc.tensor.matmul(po2[:], ones_t[:], trecs[h][:],
                                 start=True, stop=True)
                rb = wpool.tile([Co, 256], f32, tag="rb", name=f"trb{h}")
                nc.vector.tensor_copy(rb[:], po2[:])
                onorm = wpool.tile([Co, 256], bf16, tag="onorm",
                                   name=f"tonorm{h}")
                nc.vector.tensor_mul(onorm[:], po[0:Co, s], rb[:])
                py = ps_o.tile([C, 256], f32, tag="po", name=f"tpy{h}")
                nc.tensor.matmul(py[:], wbf_t[:], onorm[:],
                                 start=True, stop=True)
                yr = wpool.tile([C, 256], f32, tag=f"yr{h}", name=f"tyr{h}")
                nc.scalar.activation(yr[:], py[:], Relu, bias=bcc_t, scale=1.0)
                nc.vector.tensor_add(
                    fin_t[:, c * 512 + h * 256:c * 512 + (h + 1) * 256],
                    yr[:], img_t[:, c * 512 + h * 256:c * 512 + (h + 1) * 256],
                )
                dq = nc.sync if h == 0 else nc.scalar
                dq.dma_start(
                    out_d[:, c * 512 + h * 256:c * 512 + (h + 1) * 256],
                    fin_t[:, c * 512 + h * 256:c * 512 + (h + 1) * 256],
                )

    nc.finalize()
    return nc


def _prepare(range_x, img, wq, bq, wk, bk, wv, bv, wc, bc,
             bn_gamma, bn_beta, bn_mean, bn_var):
    """Build (or fetch) the Bass program and the 8 per-core input maps."""
    import sys
    if "/opt/trn_rl_repo" not in sys.path:
        sys.path.insert(0, "/opt/trn_rl_repo")
    import ml_dtypes

    range_x = np.asarray(range_x, np.float32)
    img = np.asarray(img, np.float32)
    wq = np.asarray(wq, np.float32)
    bq = np.asarray(bq, np.float32)
    wk = np.asarray(wk, np.float32)
    wv = np.asarray(wv, np.float32)
    bv = np.asarray(bv, np.float32)
    wc = np.asarray(wc, np.float32)
    bc = np.asarray(bc, np.float32)
    bn_gamma = np.asarray(bn_gamma, np.float32)
    bn_beta = np.asarray(bn_beta, np.float32)
    bn_mean = np.asarray(bn_mean, np.float32)
    bn_var = np.asarray(bn_var, np.float32)

    B, C, W, H = range_x.shape
    N = W * H
    NQ = N // 2
    Co = wq.shape[0]

    # Host-side weight folding (all tiny).
    inv = bn_gamma / np.sqrt(bn_var + BN_EPS)
    wcp = inv[:, None] * wc                                   # [C, Co]
    bcc = inv * bc + bn_beta - bn_mean * inv + wcp @ bv       # [C]
    at = wk.T @ wq                                            # lhsT for P-gen
    wvg = np.concatenate([wv.T, (wk.T @ bq)[:, None]], axis=1)  # [C, Co+1]

    w16 = np.concatenate([at, wvg], axis=1).astype(np.float16)
    wbf = wcp.T.astype(ml_dtypes.bfloat16)                    # [Co, C]
    x32 = bcc[:, None].astype(np.float32)

    key = (C, N, NQ, Co)
    if key not in _CACHE:
        _CACHE[key] = _build_program(C, N, NQ, Co)
    nc = _CACHE[key]

    n_cores = 8
    in_maps = []
    for core in range(n_cores):
        b, h = core // 2, core % 2
        im = img[b].reshape(C, N)
        # rotate keys so the query half is at columns 0:NQ (softmax is
        # permutation-invariant over keys; residual slice becomes fixed)
        rot = np.concatenate([im[:, h * NQ:], im[:, :h * NQ]], axis=1)
        in_maps.append({
            "w16": w16,
            "img16": rot.astype(np.float16),
            "rng16": range_x[b].reshape(C, N)[:, h * NQ:(h + 1) * NQ]
                     .astype(np.float16),
            "wbf": wbf,
            "x32": x32,
        })

    return nc, in_maps, (B, C, W, H, N, NQ)


def kernel(range_x, img, wq, bq, wk, bk, wv, bv, wc, bc,
           bn_gamma, bn_beta, bn_mean, bn_var):
    import sys
    if "/opt/trn_rl_repo" not in sys.path:
        sys.path.insert(0, "/opt/trn_rl_repo")
    from concourse.bass_utils import run_bass_kernel_spmd

    nc, in_maps, (B, C, W, H, N, NQ) = _prepare(
        range_x, img, wq, bq, wk, bk, wv, bv, wc, bc,
        bn_gamma, bn_beta, bn_mean, bn_var)

    global _LAST_RESULTS
    _LAST_RESULTS = run_bass_kernel_spmd(nc, in_maps, list(range(8)))
    res = _LAST_RESULTS.results

    out = np.empty((B, C, N), np.float32)
    for core in range(8):
        b, h = core // 2, core % 2
        out[b, :, h * NQ:(h + 1) * NQ] = res[core]["out"]
    return out.reshape(B, C, W, H)
